# revision 11
# baseline (speedup 1.0000x reference)
"""Cross-attention kernel v7 for Trainium2, data-parallel over batch across 8 cores.

v3 vs v2 (see kernel_v2.py docstring for the v2 baseline):
  - DMA order: Q/K fp8 activations+weights first so the PE starts ~2us in;
    wp/bias/V-path tensors follow.
  - Q/K projection PSUM moved into the PV pool (idle during phase 1) with
    ONE [96,2,512] DVE copy per (proj, head) instead of 32 small copies
    split across ACT+DVE. ACT now runs exps only -- the exp stream starts
    as soon as head 0's scores land.
  - Explicit slot scheduling: the 64 score tiles pace the kernel (one ACT
    exp each); between them the PE stream carries fillers -- V-projection
    chains (dh-major so PV(h0..3) only needs the first 8) spread over heads
    0-3, PV(h-2) matmul pairs spread over a head's 8 slots, and heads 5 AND
    6's PV both ride head 7's slots so the tail is only PV(h7)+norm+proj.
  - PSUM: scores 2x2 banks, pv/qkproj 1x2, proj+V 2x1 = 8 banks.
"""

import sys

if "/opt/trn_rl_repo" not in sys.path:
    sys.path.insert(0, "/opt/trn_rl_repo")

import ml_dtypes
import numpy as np

import concourse.bass as bass
import concourse.mybir as mybir
from concourse import bacc
from concourse import library_config
import concourse.tile as tile
from concourse.bass_utils import run_bass_kernel_spmd

F32 = mybir.dt.float32
BF16 = mybir.dt.bfloat16
F8 = mybir.dt.float8e4

N, S, D = 8, 1024, 768
H, HD = 8, 96
P = 128
C = D // P
SC = S // P
W_SCALE = 16.0
SCALE = 1.0 / (float(np.sqrt(D)) * W_SCALE * W_SCALE)
N_CORES = 8
EST_BUFS = 24


def build_program(reps=1):
    nc = bacc.Bacc(None, target_bir_lowering=False)

    qT8 = nc.dram_tensor("qT8", [D, S], F8, kind="ExternalInput")
    kvT8 = nc.dram_tensor("kvT8", [D, S], F8, kind="ExternalInput")
    kvT = nc.dram_tensor("kvT", [D, S], BF16, kind="ExternalInput")
    wqT8 = nc.dram_tensor("wqT8", [D, D], F8, kind="ExternalInput")
    wkT8 = nc.dram_tensor("wkT8", [D, D], F8, kind="ExternalInput")
    wvT = nc.dram_tensor("wvT", [D, D], BF16, kind="ExternalInput")
    wpT = nc.dram_tensor("wpT", [D, D], BF16, kind="ExternalInput")
    bias = nc.dram_tensor("bias", [1, D], F32, kind="ExternalInput")
    out = nc.dram_tensor("out", [S, D], F32, kind="ExternalOutput")

    with tile.TileContext(nc) as tc:
        persist = tc.alloc_tile_pool(name="persist", bufs=1)
        QT = persist.tile([HD, H, S], BF16, tag="QT")
        KT = persist.tile([HD, H, S], BF16, tag="KT")
        V = persist.tile([P, SC, H, HD + 1], BF16, tag="V")
        outhT = persist.tile([HD, H, S], BF16, tag="outhT")
        qa8 = persist.tile([P, C, S], F8, tag="qa8")
        kva8 = persist.tile([P, C, S], F8, tag="kva8")
        kva = persist.tile([P, C, S], BF16, tag="kva")
        wq8 = persist.tile([P, C, D], F8, tag="wq8")
        wk8 = persist.tile([P, C, D], F8, tag="wk8")
        wv_t = persist.tile([P, C, D], BF16, tag="wv")
        wp_t = persist.tile([HD, H, D], BF16, tag="wp")
        bias_t = persist.tile([1, D], F32, tag="bias")
        bias_bc = persist.tile([P, D], F32, tag="bias_bc")

        with (
            tc.tile_pool(name="expst", bufs=EST_BUFS) as estpool,
            tc.tile_pool(name="smmisc", bufs=2) as mpool,
            tc.tile_pool(name="osb", bufs=3) as opool,
            tc.tile_pool(name="stpsum", bufs=2, space="PSUM") as stpsum,
            tc.tile_pool(name="pvpsum", bufs=2, space="PSUM") as pvpsum,
            tc.tile_pool(name="mppsum", bufs=2, space="PSUM") as mppsum,
        ):
            for _rep in range(reps):
                nc.gpsimd.load_library(library_config.attn)
                # Pre-warm the Exp activation table during the DMA lead-in
                # so the 1.3us table load is off the first score tile.
                warm = mpool.tile([1, 8], F32, tag="warm")
                warm_o = mpool.tile([1, 8], BF16, tag="warm_o")
                nc.vector.memset(warm[:], 0.0)
                nc.scalar.activation(
                    warm_o[:], warm[:], mybir.ActivationFunctionType.Exp, scale=1.0
                )
                # Q/K fp8 path first: these gate the first matmuls.
                for c in range(C):
                    nc.sync.dma_start(qa8[:, c], qT8[c * P:(c + 1) * P, :])
                    nc.sync.dma_start(wq8[:, c], wqT8[c * P:(c + 1) * P, :])
                # Single rearranged DMAs for the later tensors: one DGE
                # descriptor-gen (~565ns) each instead of six, so the SP queue
                # frees up ~15us earlier for the Q-path-first pipeline.
                nc.sync.dma_start(kva8[:], kvT8.rearrange("(c p) s -> p c s", p=P))
                nc.sync.dma_start(wk8[:], wkT8.rearrange("(c p) o -> p c o", p=P))
                nc.sync.dma_start(kva[:], kvT.rearrange("(c p) s -> p c s", p=P))
                nc.sync.dma_start(wv_t[:], wvT.rearrange("(c p) o -> p c o", p=P))
                nc.sync.dma_start(wp_t[:], wpT.rearrange("(h d) o -> d h o", d=HD))
                nc.sync.dma_start(bias_t[:], bias[:, :])
                nc.vector.memset(V[:, :, :, HD], 1.0)
                nc.gpsimd.partition_broadcast(bias_bc[:], bias_t[:], channels=P)

                # ====== Phase 1: Q/K projections (fp8 DoubleRow) ======
                # Chains for both S-halves land in one pv-pool tile; a single
                # [96, 2, 512] DVE copy moves them to QT/KT. ACT never touches
                # these, so the exp stream owns ACT from the first score tile.
                for h in range(H):
                    for w8, acts, dstT in ((wq8, qa8, QT), (wk8, kva8, KT)):
                        for sh in range(2):
                            ps = pvpsum.tile([HD + 1, 512], F32, tag="pv")
                            for cp in range(C // 2):
                                nc.tensor.matmul(
                                    ps[0:HD],
                                    w8[:, 2 * cp:2 * cp + 2, h * HD:(h + 1) * HD],
                                    acts[:, 2 * cp:2 * cp + 2, sh * 512:(sh + 1) * 512],
                                    start=(cp == 0),
                                    stop=(cp == C // 2 - 1),
                                    perf_mode=mybir.MatmulPerfMode.DoubleRow,
                                )
                            nc.vector.tensor_copy(
                                dstT[:, h, sh * 512:(sh + 1) * 512], ps[0:HD]
                            )

                # ====== Phase 2: slot-scheduled attention ======
                est_tiles = {}

                def emit_scores(h, kc):
                    st = stpsum.tile([P, 2, 512], F32, tag="st")
                    for qh in range(2):
                        nc.tensor.matmul(
                            st[:, qh],
                            KT[:, h, kc * P:(kc + 1) * P],
                            QT[:, h, qh * 512:(qh + 1) * 512],
                            start=True,
                            stop=True,
                        )
                    est = estpool.tile([P, 2, 512], BF16, tag="est")
                    nc.scalar.activation(
                        est[:], st[:], mybir.ActivationFunctionType.Exp, scale=SCALE
                    )
                    est_tiles[(h, kc)] = est

                def emit_vproj(sc, dh):
                    ps = mppsum.tile([P, 384], F32, tag="mp")
                    for c in range(C):
                        nc.tensor.matmul(
                            ps[:],
                            kva[:, c, sc * P:(sc + 1) * P],
                            wv_t[:, c, dh * 384:(dh + 1) * 384],
                            start=(c == 0),
                            stop=(c == C - 1),
                        )
                    nc.vector.tensor_copy(
                        V[:, sc, dh * 4:(dh + 1) * 4, 0:HD],
                        ps[:].rearrange("p (h d) -> p h d", d=HD),
                    )

                # PV is emitted as per-slot matmul pairs; chain state per head.
                pv_state = {}

                def start_pv(h):
                    pv_state[h] = {"i": 0, "po": {}}

                def emit_pv_mms(h, count):
                    stt = pv_state[h]
                    for _ in range(count):
                        i = stt["i"]
                        if i >= 16:
                            return
                        qh, kc = divmod(i, SC)
                        if kc == 0:
                            stt["po"][qh] = pvpsum.tile(
                                [HD + 1, 512], F32, tag="pv", name=f"po_{h}_{qh}"
                            )
                        po = stt["po"][qh]
                        nc.tensor.matmul(
                            po[:],
                            V[:, kc, h, :],
                            est_tiles[(h, kc)][:, qh],
                            start=(kc == 0),
                            stop=(kc == SC - 1),
                        )
                        stt["i"] = i + 1
                        if kc == SC - 1:
                            recip = mpool.tile(
                                [1, 512], BF16, tag="recip", name=f"recip_{h}_{qh}"
                            )
                            with nc.allow_low_precision(
                                reason="recip feeds a broadcast multiplier in bf16"
                            ):
                                nc.vector.reciprocal(recip[:], po[HD:HD + 1])
                            bc = mpool.tile(
                                [HD, 512], BF16, tag="bc", name=f"bc_{h}_{qh}"
                            )
                            nc.gpsimd.partition_broadcast(bc[:], recip[:], channels=HD)
                            nc.vector.tensor_mul(
                                outhT[:, h, qh * 512:(qh + 1) * 512],
                                po[0:HD], bc[:],
                            )

                # V jobs dh-major: first 8 produce the V halves PV(h0..3) needs.
                vjobs = [(sc, dh) for dh in range(2) for sc in range(SC)]
                vj = 0
                for h in range(H):
                    for kc in range(SC):
                        emit_scores(h, kc)
                        if h < 4 and kc % 2 == 0 and vj < len(vjobs):
                            emit_vproj(*vjobs[vj])
                            vj += 1
                        if 2 <= h <= 5:
                            hh = h - 2
                            if hh not in pv_state:
                                start_pv(hh)
                            emit_pv_mms(hh, 2)
                        elif h == 6:
                            if kc < 4:
                                if 4 not in pv_state:
                                    start_pv(4)
                                emit_pv_mms(4, 4)
                            else:
                                if 5 not in pv_state:
                                    start_pv(5)
                                emit_pv_mms(5, 4)
                        elif h == 7:
                            if kc < 4:
                                if 6 not in pv_state:
                                    start_pv(6)
                                emit_pv_mms(6, 4)
                            if kc >= 1:
                                if 7 not in pv_state:
                                    start_pv(7)
                                emit_pv_mms(7, 1)
                    if 2 <= h <= 5:
                        emit_pv_mms(h - 2, 16)
                    elif h == 6:
                        emit_pv_mms(4, 16)
                        emit_pv_mms(5, 16)
                # tail: finish PV(7) qh0 (1 mm), then qh1 while norm(qh0)
                # and the first half of the projection overlap.
                emit_pv_mms(6, 16)
                if 7 not in pv_state:
                    start_pv(7)
                emit_pv_mms(7, 16)

                # ====== Phase 3: output projection ======
                for qc in range(SC):
                    ot = opool.tile([P, D], F32, tag="ot")
                    for oh in range(2):
                        ps = mppsum.tile([P, 384], F32, tag="mp")
                        for h in range(H):
                            nc.tensor.matmul(
                                ps[:],
                                outhT[:, h, qc * P:(qc + 1) * P],
                                wp_t[:, h, oh * 384:(oh + 1) * 384],
                                start=(h == 0),
                                stop=(h == H - 1),
                            )
                        nc.vector.tensor_add(
                            ot[:, oh * 384:(oh + 1) * 384], ps[:],
                            bias_bc[:, oh * 384:(oh + 1) * 384],
                        )
                        nc.sync.dma_start(
                            out[qc * P:(qc + 1) * P, oh * 384:(oh + 1) * 384],
                            ot[:, oh * 384:(oh + 1) * 384],
                        )

        persist.release()

    nc.compile()
    return nc


_NC_CACHE = {}


def _get_nc(reps=1):
    if reps not in _NC_CACHE:
        _NC_CACHE[reps] = build_program(reps)
    return _NC_CACHE[reps]


def _bf16(x):
    return np.ascontiguousarray(np.asarray(x, np.float32).astype(ml_dtypes.bfloat16))


def _f8(x):
    return np.ascontiguousarray(np.asarray(x, np.float32).astype(ml_dtypes.float8_e4m3))


def make_in_maps(q, kv, wq, wk, wv, w_proj, b_proj):
    q = np.asarray(q, np.float32)
    kv = np.asarray(kv, np.float32)
    qT8 = _f8(q.transpose(0, 2, 1))
    kvT8 = _f8(kv.transpose(0, 2, 1))
    kvT = _bf16(kv.transpose(0, 2, 1))
    wqT8 = _f8(np.asarray(wq, np.float32).T * W_SCALE)
    wkT8 = _f8(np.asarray(wk, np.float32).T * W_SCALE)
    wvT = _bf16(np.asarray(wv, np.float32).T)
    wpT = _bf16(np.asarray(w_proj, np.float32).T)
    b2d = np.ascontiguousarray(np.asarray(b_proj, np.float32).reshape(1, D))
    return [
        {
            "qT8": qT8[i], "kvT8": kvT8[i], "kvT": kvT[i],
            "wqT8": wqT8, "wkT8": wkT8, "wvT": wvT, "wpT": wpT,
            "bias": b2d,
        }
        for i in range(N)
    ]


def run(in_maps, trace=False, **kwargs):
    nc = _get_nc()
    return run_bass_kernel_spmd(nc, in_maps, list(range(N_CORES)), trace=trace, **kwargs)


def kernel(q, kv, wq, wk, wv, w_proj, b_proj):
    in_maps = make_in_maps(q, kv, wq, wk, wv, w_proj, b_proj)
    res = run(in_maps)
    return np.stack([res.results[i]["out"] for i in range(N_CORES)]).astype(np.float32)
